# revision 1
# baseline (speedup 1.0000x reference)
"""Deformable conv block (offset conv -> bilinear sampling -> 3x3 deform conv
-> BatchNorm + ReLU) on 8 Trainium2 NeuronCores.

Sharding: data-parallel over (image-pair, row-quarter). Core c handles images
(2*(c//4), 2*(c//4)+1) stacked on the partition dim (2 x 64 channels = 128
partitions), output rows [32*(c%4), 32*(c%4)+32).

Algorithm (C-layout, channels on partitions, all spatial shifts on free dim):
  off = conv3x3(x, offset_w) + b           (9 accumulating PE matmuls / image)
  per tap t, the bilinear sample is expanded exactly as a separable 5-term
  triangle-kernel window (offsets are < 2 in magnitude by construction):
    samp_t(c,p) = sum_vy sum_vx tri(dy_t(p)-vy) * tri(dx_t(p)-vx)
                  * x(c, p + (ti-1+vy)*W + (tj-1+vx))
  with tri(u) = relu(1-|u|).  Out-of-image samples are handled by zero-padding
  x (matches the reference's `valid` masking).  The per-position fields
  tri(dy-v) must be replicated across the 64 channel partitions: done with a
  0/1 selector matmul on the PE (exact).
  V-stage: V = sum_vy tY_vy .* x(row-shift)    (DVE, shifts = free-dim offsets)
  H-stage: S = sum_vx tX_vx .* V(col-shift)    (DVE)
  einsum:  acc += W_t.T @ S_t                  (PE, PSUM accumulation)
  BN: per-channel sums via free-dim reduce, 8-core AllReduce, affine+ReLU
  fused into one ACT pass per image block.
"""
import os
import numpy as np

C, K2, H, W, B = 64, 9, 128, 128, 4
NCORES = 8
RPC = 32          # output rows per core
QR = 8            # rows per quarter-chunk
NQ = RPC // QR    # 4 quarters
PITCH = 134       # padded col pitch; col index = 3 + w, w in [-3, 131)
XROWS = RPC + 6   # 3-row halo each side
EPS = 1e-5
NPOS = float(B * H * W)

_CACHE = {}


def _build_program(debug_dump=False):
    DBG_T = int(os.environ.get("DBG_T", "4"))
    DBG_SINGLE = bool(int(os.environ.get("DBG_SINGLE", "0")))
    STAGE = int(os.environ.get("KM_STAGE", "5"))
    from contextlib import ExitStack
    import concourse.bass as bass
    import concourse.tile as tile
    from concourse import bacc, mybir

    f32 = mybir.dt.float32
    f32r = mybir.dt.float32r
    AF = mybir.ActivationFunctionType
    OP = mybir.AluOpType

    nc = bacc.Bacc(
        "TRN2",
        target_bir_lowering=False,
        debug=False,
        enable_asserts=False,
        num_devices=NCORES,
    )

    for cval in (-2.0, -1.0, 2.0):
        _ct = nc.alloc_sbuf_tensor(f"const-f32-{cval}", [128, 1], f32)
        nc.gpsimd.memset(_ct.ap(), cval)
        nc.const_aps.aps[(f32, cval)] = _ct.ap()
    nc.all_engine_barrier()

    xs_d = nc.dram_tensor("xs", (128, XROWS * PITCH), f32, kind="ExternalInput")
    ow_d = nc.dram_tensor("ow", (128, K2 * 18), f32, kind="ExternalInput")
    ob_d = nc.dram_tensor("ob", (50, 1), f32, kind="ExternalInput")
    sel_d = nc.dram_tensor("sel", (50, 18 * 128), f32, kind="ExternalInput")
    wt_d = nc.dram_tensor("wt", (128, K2 * 64), f32, kind="ExternalInput")
    gb_d = nc.dram_tensor("gb", (64, 2), f32, kind="ExternalInput")
    out_d = nc.dram_tensor("out", (128, RPC * 128), f32, kind="ExternalOutput")
    stats_in_d = nc.dram_tensor("stats_in", (128, 2), f32, kind="Internal")
    stats_sh_d = nc.dram_tensor(
        "stats_sh", (128, 2), f32, kind="Internal", addr_space="Shared"
    )
    if debug_dump:
        dbg_offc_d = nc.dram_tensor("dbg_offc", (50, QR * 128), f32, kind="ExternalOutput")
        dbg_offr_d = nc.dram_tensor("dbg_offr", (128, 2 * QR * PITCH), f32, kind="ExternalOutput")
        dbg_fld_d = nc.dram_tensor("dbg_fld", (128, 10 * QR * PITCH), f32, kind="ExternalOutput")
        dbg_vv_d = nc.dram_tensor("dbg_vv", (128, QR * PITCH), f32, kind="ExternalOutput")
        dbg_hs_d = nc.dram_tensor("dbg_hs", (128, K2 * QR * 128), f32, kind="ExternalOutput")
        dbg_pre_d = nc.dram_tensor("dbg_pre", (128, RPC * 128), f32, kind="ExternalOutput")

    with tile.TileContext(nc) as tc, ExitStack() as ctx:
        consts = ctx.enter_context(tc.tile_pool(name="consts", bufs=1))
        main = ctx.enter_context(tc.tile_pool(name="main", bufs=1))
        offc_pool = ctx.enter_context(tc.tile_pool(name="offc", bufs=2))
        offr_pool = ctx.enter_context(tc.tile_pool(name="offr", bufs=2))
        fld_pool = ctx.enter_context(tc.tile_pool(name="fld", bufs=1))
        v_pool = ctx.enter_context(tc.tile_pool(name="vv", bufs=1))
        tmp_pool = ctx.enter_context(tc.tile_pool(name="tmp", bufs=1))
        h_pool = ctx.enter_context(tc.tile_pool(name="hs", bufs=2))
        stat_pool = ctx.enter_context(tc.tile_pool(name="stat", bufs=1))
        ps_off = ctx.enter_context(tc.tile_pool(name="ps_off", bufs=1, space="PSUM"))
        ps_sel = ctx.enter_context(tc.tile_pool(name="ps_sel", bufs=2, space="PSUM"))
        ps_acc = ctx.enter_context(tc.tile_pool(name="ps_acc", bufs=1, space="PSUM"))

        xs = consts.tile([128, XROWS * PITCH], f32, tag="xs")
        ow = consts.tile([128, K2 * 18], f32, tag="ow")
        ob = consts.tile([50, 1], f32, tag="ob")
        sel = consts.tile([50, 18 * 128], f32, tag="sel")
        wt = consts.tile([128, K2 * 64], f32, tag="wt")
        gb = consts.tile([64, 2], f32, tag="gb")
        nc.sync.dma_start(xs[:], xs_d.ap())
        nc.sync.dma_start(ow[:], ow_d.ap())
        nc.sync.dma_start(ob[:], ob_d.ap())
        nc.sync.dma_start(sel[:], sel_d.ap())
        nc.sync.dma_start(wt[:], wt_d.ap())
        nc.sync.dma_start(gb[:], gb_d.ap())

        out_pre = main.tile([128, RPC * 128], f32, tag="out_pre")
        if STAGE < 3:
            nc.vector.memset(out_pre[:], 0.0)
        xs3 = xs[:].rearrange("p (r c) -> p r c", r=XROWS)

        for q in range(NQ):
            # ---- offset conv for this quarter: off_C (50, QR*128) ----
            offp = [ps_off.tile([50, 512], f32, tag=f"offp{h2}", name=f"offp{h2}") for h2 in range(2)]
            for img in range(2):
                pb = img * 64          # x partition base
                obb = img * 32         # off_C row base (0 / 32)
                for t9 in range(K2):
                    ti, tj = t9 // 3, t9 % 3
                    for h2 in range(2):
                        rhs = xs3[
                            pb : pb + 64,
                            8 * q + 2 + ti + 4 * h2 : 8 * q + 2 + ti + 4 * h2 + 4,
                            2 + tj : 2 + tj + 128,
                        ]
                        nc.tensor.matmul(
                            offp[h2][obb : obb + 18, :],
                            ow[pb : pb + 64, t9 * 18 : (t9 + 1) * 18],
                            rhs,
                            start=(t9 == 0),
                            stop=(t9 == K2 - 1),
                        )
            offc = offc_pool.tile([50, QR * 128], f32, tag="offc")
            nc.gpsimd.memset(offc[:], 0.0)
            for img in range(2):
                obb = img * 32
                for h2 in range(2):
                    nc.scalar.activation(
                        offc[obb : obb + 18, h2 * 512 : (h2 + 1) * 512],
                        offp[h2][obb : obb + 18, :],
                        AF.Identity, bias=ob[obb : obb + 18, :], scale=1.0,
                    )

            acc = [
                ps_acc.tile([128, 512], f32, tag=f"acc{i}", name=f"acc{i}")
                for i in range(4)
            ]

            for t in range(K2 if STAGE >= 2 else 0):
                ki, kj = t // 3, t % 3
                # ---- replicate dy_t, dx_t across channel partitions ----
                offr = offr_pool.tile([128, 2, QR, PITCH], f32, tag="offr")
                nc.gpsimd.memset(offr[:], 0.0)
                for dyx in range(2):
                    for h2 in range(2):
                        ps = ps_sel.tile([128, 512], f32, tag="ps_sel")
                        nc.tensor.matmul(
                            ps[:],
                            sel[:, (2 * t + dyx) * 128 : (2 * t + dyx + 1) * 128],
                            offc[:, h2 * 512 : (h2 + 1) * 512],
                            start=True, stop=True,
                        )
                        nc.scalar.copy(
                            offr[:, dyx, 4 * h2 : 4 * h2 + 4, 3 : 131],
                            ps[:].rearrange("p (r c) -> p r c", r=4),
                        )
                # ---- tri fields: fld[:,0:5]=tY_v, fld[:,5:10]=tX_v ----
                fld = fld_pool.tile([128, 5, 2, QR, PITCH], f32, tag="fld")
                for i, v in enumerate((-2.0, -1.0, 0.0, 1.0, 2.0)):
                    f = fld[:, i, :, :, :]
                    nc.scalar.activation(f, offr[:], AF.Abs, bias=-v, scale=1.0)
                    nc.scalar.activation(f, f, AF.Relu, bias=1.0, scale=-1.0)
                if debug_dump and q == 0 and t == DBG_T:
                    nc.sync.dma_start(dbg_offr_d.ap(), offr[:].rearrange("p a r c -> p (a r c)"))
                    nc.sync.dma_start(dbg_fld_d.ap(), fld[:].rearrange("p a b r c -> p (a b r c)"))
                if STAGE < 3:
                    continue
                # ---- bilinear: per col-shift group, row-combine anchored at
                # output; each group's weighted patch goes straight into the
                # PSUM-accumulated einsum. Groups j=0/4 run on GpSimd, rest on
                # the Vector engine (DVE is the kernel bottleneck). ----
                hs_parts = []
                for j, vx in enumerate((-2, -1, 0, 1, 2)):
                    eng = nc.vector
                    s = kj - 1 + vx
                    vv = v_pool.tile([128, QR, 128], f32, tag=f"vv{j}", name=f"vv{j}")
                    tmp = tmp_pool.tile([128, QR, 128], f32, tag=f"tmp{j}", name=f"tmp{j}")
                    hsP = h_pool.tile([128, QR, 128], f32, tag=f"hs{j}", name=f"hs{j}", bufs=1)
                    for i, vy in enumerate((-2, -1, 0, 1, 2)):
                        r0 = 8 * q + 2 + ki + vy
                        xv = xs3[:, r0 : r0 + QR, 3 + s : 131 + s]
                        ty = fld[:, i, 0, :, 3:131]
                        if i == 0:
                            eng.tensor_tensor(vv[:], ty, xv, OP.mult)
                        else:
                            eng.tensor_tensor(tmp[:], ty, xv, OP.mult)
                            eng.tensor_tensor(vv[:], vv[:], tmp[:], OP.add)
                    tx = fld[:, j, 1, :, 3:131]
                    eng.tensor_tensor(hsP[:], tx, vv[:], OP.mult)
                    hs_parts.append(hsP)
                hs = h_pool.tile([128, QR, 128], f32, tag="hsacc", name="hsacc", bufs=2)
                nc.vector.tensor_tensor(hs[:], hs_parts[0][:], hs_parts[1][:], OP.add)
                nc.vector.tensor_tensor(hs[:], hs[:], hs_parts[2][:], OP.add)
                nc.vector.tensor_tensor(hs[:], hs[:], hs_parts[3][:], OP.add)
                nc.vector.tensor_tensor(hs[:], hs[:], hs_parts[4][:], OP.add)
                for img in range(2):
                    pb = img * 64
                    for h2 in range(2):
                        nc.tensor.matmul(
                            acc[2 * img + h2][pb : pb + 64, :],
                            wt[pb : pb + 64, t * 64 : (t + 1) * 64],
                            hs[pb : pb + 64, :, :]
                            .rearrange("p r c -> p (r c)")[:, h2 * 512 : (h2 + 1) * 512],
                            start=(t == 0),
                            stop=(t == K2 - 1),
                        )
            for img in range(2 if STAGE >= 3 else 0):
                pb = img * 64
                for h2 in range(2):
                    nc.scalar.copy(
                        out_pre[pb : pb + 64, q * 1024 + h2 * 512 : q * 1024 + (h2 + 1) * 512],
                        acc[2 * img + h2][pb : pb + 64, :],
                    )
            if debug_dump and q == 0:
                nc.sync.dma_start(dbg_offc_d.ap(), offc[:])


        do_bn = STAGE >= 4
        # ---- BatchNorm stats ----
        if not do_bn:
            nc.sync.dma_start(out_d.ap(), out_pre[:])
        if do_bn:
            sums = stat_pool.tile([128, 2], f32, tag="sums")
            scr = main.tile([128, RPC * 128], f32, tag="scr")
            nc.vector.tensor_reduce(sums[:, 0:1], out_pre[:], mybir.AxisListType.X, OP.add)
            nc.scalar.activation(scr[:], out_pre[:], AF.Square)
            nc.vector.tensor_reduce(sums[:, 1:2], scr[:], mybir.AxisListType.X, OP.add)
            NO_CC = bool(int(os.environ.get("KM_NO_CC", "0")))
            nc.sync.dma_start(stats_in_d.ap(), sums[:])
            if not NO_CC:
                nc.gpsimd.collective_compute(
                    "AllReduce", OP.add, [list(range(NCORES))],
                    ins=[stats_in_d.ap()], outs=[stats_sh_d.ap()],
                )
            stats_src = stats_in_d if NO_CC else stats_sh_d
            tot_a = stat_pool.tile([64, 2], f32, tag="tot_a")
            tot_b = stat_pool.tile([64, 2], f32, tag="tot_b")
            nc.sync.dma_start(tot_a[:], stats_src.ap()[0:64, :])
            nc.sync.dma_start(tot_b[:], stats_src.ap()[64:128, :])
            tot64 = stat_pool.tile([64, 2], f32, tag="tot64")
            nc.vector.tensor_tensor(tot64[:], tot_a[:], tot_b[:], OP.add)
            fin = stat_pool.tile([64, 8], f32, tag="fin")
            mu = fin[:, 0:1]; ex2 = fin[:, 1:2]; m2 = fin[:, 2:3]; var = fin[:, 3:4]
            inv = fin[:, 4:5]; rstd = fin[:, 5:6]; sc = fin[:, 6:7]; tc_ = fin[:, 7:8]
            npos_eff = NPOS / NCORES if NO_CC else NPOS
            nc.vector.tensor_scalar_mul(mu, tot64[:, 0:1], 1.0 / npos_eff)
            nc.vector.tensor_scalar_mul(ex2, tot64[:, 1:2], 1.0 / npos_eff)
            nc.vector.tensor_tensor(m2, mu, mu, OP.mult)
            nc.vector.tensor_tensor(var, ex2, m2, OP.subtract)
            nc.vector.tensor_scalar_add(var, var, EPS)
            nc.vector.reciprocal(inv, var)
            nc.scalar.activation(rstd, inv, AF.Sqrt)
            nc.vector.tensor_tensor(sc, rstd, gb[:, 0:1], OP.mult)
            nc.vector.tensor_tensor(tc_, mu, sc, OP.mult)
            nc.vector.tensor_tensor(tc_, gb[:, 1:2], tc_, OP.subtract)
            if debug_dump:
                nc.sync.dma_start(dbg_pre_d.ap(), out_pre[:])
            st = stat_pool.tile([128, 2], f32, tag="st")
            nc.sync.dma_start(st[0:64, :], fin[:, 6:8])
            nc.sync.dma_start(st[64:128, :], fin[:, 6:8])
            nc.vector.scalar_tensor_tensor(
                out_pre[:], out_pre[:], st[:, 0:1],
                st[:, 1:2].broadcast_to([128, RPC * 128]),
                OP.mult, OP.add,
            )
            nc.vector.tensor_scalar_max(out_pre[:], out_pre[:], 0.0)
            nc.sync.dma_start(out_d.ap(), out_pre[:])

    nc.compile()
    return nc


def _shard_inputs(x, offset_w, offset_b, dcn_w, gamma, beta):
    """Build the 8 per-core input maps (all float32 numpy)."""
    x = np.asarray(x, np.float32)
    ow_full = np.asarray(offset_w, np.float32)
    ob_full = np.asarray(offset_b, np.float32)
    wt_full = np.asarray(dcn_w, np.float32)

    ow1 = ow_full.transpose(1, 2, 3, 0).reshape(64, K2 * 18)
    ow = np.concatenate([ow1, ow1], axis=0).copy()  # duplicated for both partition halves
    wt1 = wt_full.transpose(1, 2, 3, 0).reshape(64, K2 * 64)
    wt = np.concatenate([wt1, wt1], axis=0).copy()  # duplicated for both partition halves
    ob = np.zeros((50, 1), np.float32)
    ob[0:18, 0] = ob_full
    ob[32:50, 0] = ob_full
    sel = np.zeros((50, 18 * 128), np.float32)
    for t in range(K2):
        for dyx in range(2):
            j = (2 * t + dyx) * 128
            sel[2 * t + dyx, j : j + 64] = 1.0
            sel[32 + 2 * t + dyx, j + 64 : j + 128] = 1.0
    gb = np.stack(
        [np.asarray(gamma, np.float32), np.asarray(beta, np.float32)], axis=1
    ).copy()

    in_maps = []
    for core in range(NCORES):
        pair, q = core // 4, core % 4
        shard = np.zeros((128, XROWS, PITCH), np.float32)
        r_lo = 32 * q - 3
        for blk in range(2):
            img = 2 * pair + blk
            g0, g1 = max(0, r_lo), min(H, r_lo + XROWS)
            shard[blk * 64 : (blk + 1) * 64, g0 - r_lo : g1 - r_lo, 3:131] = x[
                img, :, g0:g1, :
            ]
        in_maps.append(
            dict(
                xs=shard.reshape(128, XROWS * PITCH).copy(),
                ow=ow, ob=ob, sel=sel, wt=wt, gb=gb,
            )
        )
    return in_maps


def kernel(x, offset_w, offset_b, dcn_w, gamma, beta):
    from concourse.bass_utils import run_bass_kernel_spmd

    if "nc" not in _CACHE:
        _CACHE["nc"] = _build_program()
    nc = _CACHE["nc"]

    in_maps = _shard_inputs(x, offset_w, offset_b, dcn_w, gamma, beta)
    res = run_bass_kernel_spmd(nc, in_maps, core_ids=list(range(NCORES)))
    out = np.zeros((B, C, H, W), np.float32)
    for core in range(NCORES):
        pair, q = core // 4, core % 4
        o = res.results[core]["out"].reshape(128, RPC, 128)
        for blk in range(2):
            out[2 * pair + blk, :, 32 * q : 32 * q + 32, :] = o[
                blk * 64 : (blk + 1) * 64
            ]
    return out

